# revision 21
# baseline (speedup 1.0000x reference)
"""Trainium2 Bass kernel for a 2-layer GRU LM step (T=35, B=128, E=200, H=512, V=10000).

Strategy (8 NeuronCores):
- Phase 0: per-core embedding gather (indirect DMA) for its B-shard (16 rows),
  PE-transpose x into [E, T*B_loc] layout; load weights (host pre-transposed).
- Phase 1: recurrence, data-parallel on batch (B_loc=16/core). Activation-
  stationary matmuls: stationary = hT [K,16] slices, moving = W.T [K, <=512].
  All biases folded in via ones-rows appended to the stationary operands.
  h1 (layer-2 output, transposed [H, 16]) is appended to an SBUF history tile
  and AllGathered across cores in 8-step chunks via DRAM.
- Phase 2: logits GEMM, vocab-sharded (1250 cols/core): stationary = gathered
  h1T tiles [128,128], moving = Wout.T slice; bias added during PSUM drain;
  output written rank-block-major, host unpermutes.
"""

import sys

for _p in ("/opt/trn_rl_repo", "/root/.axon_site/_ro/trn_rl_repo"):
    if _p not in sys.path:
        sys.path.insert(0, _p)

import numpy as np

import concourse.bass as bass
import concourse.mybir as mybir
import concourse.tile as tile
from concourse import bacc
from concourse.bass_utils import run_bass_kernel_spmd
from concourse.masks import make_identity

T, B, E, H, L, V = 35, 128, 200, 512, 2, 10000
NC = 8
BL = B // NC            # 16 local batch rows per core
TBL = T * BL            # 560
VL = V // NC            # 1250 vocab cols per core
GTILES = 5              # ceil(TBL/128) gather tiles
KH = H // 128           # 4 K-chunks over H
F32 = mybir.dt.float32
F32R = mybir.dt.float32r
I32 = mybir.dt.int32

# AllGather chunking: 8 steps per chunk -> cols of 128 in the hT history
CHUNKS = [(c * 8, min(8, T - c * 8)) for c in range((T + 7) // 8)]  # [(t0, nsteps)]


def _np(x):
    return np.asarray(x)


def build_kernel(sim=False):
    nc = bacc.Bacc("TRN2", target_bir_lowering=False, debug=False,
                   num_devices=(1 if sim else NC))

    # ---- I/O ----
    idx_in = nc.dram_tensor("idx_in", [GTILES, 128, 1], I32, kind="ExternalInput")
    emb_in = nc.dram_tensor("emb_in", [V, E], F32, kind="ExternalInput")
    wx0_in = nc.dram_tensor("wx0_in", [E + 1, 3 * H], F32R, kind="ExternalInput")
    wh0rz_in = nc.dram_tensor("wh0rz_in", [H, 2 * H], F32R, kind="ExternalInput")
    wh0c_in = nc.dram_tensor("wh0c_in", [H, H], F32R, kind="ExternalInput")
    wx1_in = nc.dram_tensor("wx1_in", [H + 1, 3 * H], F32R, kind="ExternalInput")
    wh1rz_in = nc.dram_tensor("wh1rz_in", [H, 2 * H], F32R, kind="ExternalInput")
    wh1c_in = nc.dram_tensor("wh1c_in", [H, H], F32R, kind="ExternalInput")
    wout_in = nc.dram_tensor("wout_in", [H, VL], F32R, kind="ExternalInput")
    bout_in = nc.dram_tensor("bout_in", [1, VL], F32, kind="ExternalInput")
    hid_in = nc.dram_tensor("hid_in", [L, BL, H], F32, kind="ExternalInput")

    logits_out = nc.dram_tensor("logits_out", [NC, TBL, VL], F32, kind="ExternalOutput")
    hfin_out = nc.dram_tensor("hfin_out", [L, BL, H], F32, kind="ExternalOutput")

    with tile.TileContext(nc) as tc:
        _body(nc, tc, idx_in, emb_in, wx0_in, wh0rz_in, wh0c_in, wx1_in,
              wh1rz_in, wh1c_in, wout_in, bout_in, hid_in, logits_out, hfin_out,
              sim=sim)
    nc.compile()
    return nc


def _body(nc, tc, idx_in, emb_in, wx0_in, wh0rz_in, wh0c_in, wx1_in,
          wh1rz_in, wh1c_in, wout_in, bout_in, hid_in, logits_out, hfin_out,
          sim=False):
    sig = mybir.ActivationFunctionType.Sigmoid
    tanh = mybir.ActivationFunctionType.Tanh
    def mm(out, lhsT, rhs, **kw):
        nc.tensor.matmul(out, lhsT=lhsT, rhs=rhs, **kw)

    from contextlib import ExitStack
    ctx = ExitStack()
    const = ctx.enter_context(tc.tile_pool(name="const", bufs=1))
    wpool = ctx.enter_context(tc.tile_pool(name="weights", bufs=1))
    spool = ctx.enter_context(tc.tile_pool(name="scratch", bufs=2))
    state = ctx.enter_context(tc.tile_pool(name="state", bufs=2))
    hist_pool = ctx.enter_context(tc.tile_pool(name="hist", bufs=1))
    psum_g = ctx.enter_context(tc.tile_pool(name="psum_g", bufs=2, space="PSUM"))
    psum_t = ctx.enter_context(tc.tile_pool(name="psum_t", bufs=2, space="PSUM"))
    dram = ctx.enter_context(tc.tile_pool(name="dram", bufs=1, space="DRAM"))

    # ---- constants ----
    ident = const.tile([128, 128], F32, name="ident")
    make_identity(nc, ident[:])
    ones16 = const.tile([1, BL], F32R, name="ones16")
    nc.vector.memset(ones16[:].bitcast(F32), 1.0)

    # ---- load weights ----
    def load_w(name, src, rows, cols):
        # rows x cols DRAM -> list of [<=128, cols] SBUF tiles per 128-row chunk
        tiles = []
        for k in range((rows + 127) // 128):
            r0, r1 = k * 128, min(rows, (k + 1) * 128)
            tl = wpool.tile([r1 - r0, cols], F32R, name=f"{name}_{k}")
            nc.sync.dma_start(out=tl[:], in_=src.ap()[r0:r1, :])
            tiles.append(tl)
        return tiles

    wx0 = load_w("wx0", wx0_in, E + 1, 3 * H)       # [128,1536],[73,1536]
    wh0rz = load_w("wh0rz", wh0rz_in, H, 2 * H)     # 4x [128,1024]
    wh0c = load_w("wh0c", wh0c_in, H, H)            # 4x [128,512]
    wx1 = load_w("wx1", wx1_in, H + 1, 3 * H)       # 4x [128,1536] + [1,1536]
    wh1rz = load_w("wh1rz", wh1rz_in, H, 2 * H)
    wh1c = load_w("wh1c", wh1c_in, H, H)
    wout = load_w("wout", wout_in, H, VL)           # 4x [128,1250]

    bias_b = const.tile([128, VL], F32, name="bias_b")
    nc.sync.dma_start(out=bias_b[:], in_=bout_in.ap()[0:1, :].to_broadcast([128, VL]))

    # ---- embedding gather + transpose to xT [E+1, TBL] ----
    xT_a = const.tile([128, GTILES * 128], F32R, name="xT_a")     # E rows 0..127
    xT_b = const.tile([128, GTILES * 128], F32R, name="xT_b")     # E rows 128..199 + ones row
    nc.vector.memset(xT_b[:].bitcast(F32), 1.0)   # row 72 stays 1.0 -> bh[0] bias via wx0 row E

    for i in range(GTILES):
        idx_sb = spool.tile([128, 1], I32, name="idx_sb")
        nc.sync.dma_start(out=idx_sb[:], in_=idx_in.ap()[i, :, :])
        x_sb = spool.tile([128, E], F32, name="x_sb")
        nc.gpsimd.indirect_dma_start(
            out=x_sb[:], out_offset=None, in_=emb_in.ap(),
            in_offset=bass.IndirectOffsetOnAxis(ap=idx_sb[:, :1], axis=0),
        )
        pt1 = psum_t.tile([128, 128], F32, name="pt1", tag="pt_tr")
        nc.tensor.transpose(out=pt1[:], in_=x_sb[:, 0:128], identity=ident[:])
        nc.vector.tensor_copy(out=xT_a[:, i * 128:(i + 1) * 128], in_=pt1[:])
        pt2 = psum_t.tile([72, 128], F32, name="pt2", tag="pt_tr")
        nc.tensor.transpose(out=pt2[:], in_=x_sb[:, 128:E], identity=ident[:])
        nc.vector.tensor_copy(out=xT_b[0:72, i * 128:(i + 1) * 128], in_=pt2[:])

    # ---- initial hidden state ----
    h_cur = []      # [16, 512] untransposed, per layer
    hT_init = []    # [128, 64] transposed (k-th chunk at cols 16k..16k+16)
    for j in range(L):
        h_sb = state.tile([BL, H], F32, name=f"hinit_{j}", bufs=1)
        nc.sync.dma_start(out=h_sb[:], in_=hid_in.ap()[j, :, :])
        h_cur.append(h_sb)
        hT = state.tile([128, 4 * BL], F32R, name=f"hTinit_{j}", bufs=1)
        pt = psum_t.tile([128, 4 * BL], F32, name="pt_init", tag="pt_tr")
        for k in range(KH):
            nc.tensor.transpose(out=pt[:, k * BL:(k + 1) * BL],
                                in_=h_sb[:, k * 128:(k + 1) * 128],
                                identity=ident[0:BL, 0:BL])
        nc.vector.tensor_copy(out=hT[:], in_=pt[:])
        hT_init.append(hT)

    # h1T history [128, TBL] per K-chunk
    hist = [hist_pool.tile([128, TBL], F32R, name=f"hist_{k}") for k in range(KH)]

    # AllGather buffers per chunk
    ag_out = []
    for ci, (t0, ns) in enumerate(CHUNKS):
        cols = ns * BL
        agi = dram.tile([KH, 128, cols], F32R, name=f"agin_{ci}")
        ago = dram.tile([NC, KH, 128, cols], F32R, name=f"agout_{ci}",
                        addr_space="Shared")
        ag_out.append((agi, ago, cols))

    def transpose_16(src_sb, name):
        """[16,512] -> [128, 64] (k-th 128-chunk of H at cols 16k..)."""
        pt = psum_t.tile([128, 4 * BL], F32, name=f"pt_{name}", tag="pt_tr")
        for k in range(KH):
            nc.tensor.transpose(out=pt[:, k * BL:(k + 1) * BL],
                                in_=src_sb[:, k * 128:(k + 1) * 128],
                                identity=ident[0:BL, 0:BL])
        dst = state.tile([128, 4 * BL], F32R, name=f"T_{name}", tag=f"T_{name}")
        nc.vector.tensor_copy(out=dst[:], in_=pt[:])
        return dst

    def gru_update(h_old, z_ap, c_sb, tag):
        """h_new = h_old + z * (c - h_old); all [16, 512] SBUF."""
        d = spool.tile([BL, H], F32, name=f"d_{tag}", tag="d")
        nc.vector.tensor_sub(out=d[:], in0=c_sb[:], in1=h_old[:])
        nc.vector.tensor_mul(out=d[:], in0=d[:], in1=z_ap)
        h_new = state.tile([BL, H], F32, name=f"h_{tag}", tag=f"h_{tag}")
        nc.vector.tensor_add(out=h_new[:], in0=h_old[:], in1=d[:])
        return h_new

    # Phase-2 logits emission for one (chunk, rank-block); interleaved into
    # the recurrence loop one AG-chunk behind so the collective has slack.
    NSL = [(0, 418), (418, 418), (836, VL - 836)]   # even n-slices >=256 for f32r

    def emit_p2_block(ci, rb):
        t0, ns = CHUNKS[ci]
        agi, ago, cols = ag_out[ci]
        hT_t = []
        for k in range(KH):
            s = spool.tile([128, cols], F32R, name=f"hTt_{k}", tag=f"hTt_{k}")
            nc.sync.dma_start(out=s[:], in_=ago[rb, k, :, :])
            hT_t.append(s)
        out_sb = spool.tile([cols, VL], F32, name="out_sb", tag="out_sb", bufs=2)
        for (n0, nw) in NSL:
            pl = psum_g.tile([cols, nw], F32, name="pl", tag="pg0", bufs=2)
            for k in range(KH):
                mm(pl[:], hT_t[k][:], rhs=wout[k][:, n0:n0 + nw],
                   start=(k == 0), stop=(k == KH - 1))
            nc.vector.tensor_add(out=out_sb[:, n0:n0 + nw], in0=pl[:],
                                 in1=bias_b[0:cols, n0:n0 + nw])
        nc.sync.dma_start(out=logits_out.ap()[rb, t0 * BL:t0 * BL + cols, :],
                          in_=out_sb[:])

    p2_emitted = set()
    p2_remaining = []

    # ---- Phase 1: recurrence ----
    h0T_prev, h1T_prev = hT_init[0], hT_init[1]
    for t in range(T):
        xcol = slice(t * BL, (t + 1) * BL)

        def h1T_ap(k):
            if t == 0:
                return h1T_prev[:, k * BL:(k + 1) * BL]
            return hist[k][:, (t - 1) * BL:t * BL]

        # --- layer 0: r,z gates ---
        p_rz0 = [psum_g.tile([BL, H], F32, name=f"prz0_{n}", tag=f"pg{n}") for n in range(2)]
        for n in range(2):
            mm(p_rz0[n][:], xT_a[:, xcol],
                             rhs=wx0[0][:, n * H:(n + 1) * H], start=True, stop=False)
            mm(p_rz0[n][:], xT_b[0:73, xcol],
                             rhs=wx0[1][:, n * H:(n + 1) * H], start=False, stop=False)
            for k in range(KH):
                mm(p_rz0[n][:], h0T_prev[:, k * BL:(k + 1) * BL],
                                 rhs=wh0rz[k][:, n * H:(n + 1) * H],
                                 start=False, stop=(k == KH - 1))
        rz0 = spool.tile([BL, 2 * H], F32, name="rz0", tag="rz", bufs=3)
        for n in range(2):
            nc.scalar.activation(rz0[:, n * H:(n + 1) * H], p_rz0[n][:], sig)

        # --- layer 0: candidate ---
        rh0 = spool.tile([BL, H], F32, name="rh0", tag="rh0")
        nc.vector.tensor_mul(out=rh0[:], in0=rz0[:, 0:H], in1=h_cur[0][:])
        rh0T = transpose_16(rh0, "rh0")
        p_c0 = psum_g.tile([BL, H], F32, name="pc0", tag="pg2")
        mm(p_c0[:], xT_a[:, xcol], rhs=wx0[0][:, 2 * H:3 * H],
                         start=True, stop=False)
        mm(p_c0[:], xT_b[0:73, xcol], rhs=wx0[1][:, 2 * H:3 * H],
                         start=False, stop=False)
        for k in range(KH):
            mm(p_c0[:], rh0T[:, k * BL:(k + 1) * BL],
                             rhs=wh0c[k][:], start=False, stop=(k == KH - 1))
        c0 = spool.tile([BL, H], F32, name="c0", tag="c", bufs=2)
        nc.scalar.activation(c0[:], p_c0[:], tanh)
        h_cur[0] = gru_update(h_cur[0], rz0[:, H:2 * H], c0, "h0")
        h0T_prev = transpose_16(h_cur[0], "h0")

        # --- layer 1: recurrent r,z matmuls first (depend only on h1[t-1]),
        # then x-side (waits on fresh h0[t]) ---
        p1 = [psum_g.tile([BL, H], F32, name=f"p1_{n}", tag=f"pg{n}") for n in range(3)]
        for n in range(2):
            for k in range(KH):
                mm(p1[n][:], h1T_ap(k),
                                 rhs=wh1rz[k][:, n * H:(n + 1) * H],
                                 start=(k == 0), stop=False)
        for n in range(3):
            for k in range(KH):
                mm(p1[n][:], h0T_prev[:, k * BL:(k + 1) * BL],
                                 rhs=wx1[k][:, n * H:(n + 1) * H],
                                 start=(n == 2 and k == 0), stop=False)
            mm(p1[n][:], ones16[:], rhs=wx1[4][:, n * H:(n + 1) * H],
                             start=False, stop=(n < 2))
        rz1 = spool.tile([BL, 2 * H], F32, name="rz1", tag="rz", bufs=3)
        for n in range(2):
            nc.scalar.activation(rz1[:, n * H:(n + 1) * H], p1[n][:], sig)

        # --- layer 1: candidate ---
        rh1 = spool.tile([BL, H], F32, name="rh1", tag="rh1")
        nc.vector.tensor_mul(out=rh1[:], in0=rz1[:, 0:H], in1=h_cur[1][:])
        rh1T = transpose_16(rh1, "rh1")
        for k in range(KH):
            mm(p1[2][:], rh1T[:, k * BL:(k + 1) * BL],
                             rhs=wh1c[k][:], start=False, stop=(k == KH - 1))
        c1 = spool.tile([BL, H], F32, name="c1", tag="c", bufs=2)
        nc.scalar.activation(c1[:], p1[2][:], tanh)
        h_cur[1] = gru_update(h_cur[1], rz1[:, H:2 * H], c1, "h1")

        # transpose h1 into the history tile
        pt = psum_t.tile([128, 4 * BL], F32, name="pt_h1", tag="pt_tr")
        for k in range(KH):
            nc.tensor.transpose(out=pt[:, k * BL:(k + 1) * BL],
                                in_=h_cur[1][:, k * 128:(k + 1) * 128],
                                identity=ident[0:BL, 0:BL])
        for k in range(KH):
            nc.vector.tensor_copy(out=hist[k][:, t * BL:(t + 1) * BL],
                                  in_=pt[:, k * BL:(k + 1) * BL])

        # --- AllGather finished chunk ---
        for ci, (t0, ns) in enumerate(CHUNKS):
            if t == t0 + ns - 1:
                agi, ago, cols = ag_out[ci]
                for k in range(KH):
                    nc.sync.dma_start(out=agi[k, :, :],
                                      in_=hist[k][:, t0 * BL:t0 * BL + cols])
                if sim:
                    nc.sync.dma_start(out=ago[0], in_=agi[:])
                else:
                    nc.gpsimd.collective_compute(
                        "AllGather", mybir.AluOpType.bypass,
                        replica_groups=[list(range(NC))],
                        ins=[agi.opt()], outs=[ago.opt()],
                    )

        # interleave one lagged phase-2 logits block per step
        ci, rb = t // 8 - 1, t % 8
        if 0 <= ci < len(CHUNKS):
            emit_p2_block(ci, rb)
            p2_emitted.add((ci, rb))

    for ci in range(len(CHUNKS)):
        for rb in range(NC):
            if (ci, rb) not in p2_emitted:
                p2_remaining.append((ci, rb))

    # final hidden state out
    for j in range(L):
        nc.sync.dma_start(out=hfin_out.ap()[j, :, :], in_=h_cur[j][:])

    # ---- Phase 2 tail: remaining logits blocks ----
    for ci, rb in p2_remaining:
        emit_p2_block(ci, rb)
    ctx.close()


_CACHED = None


def _get_kernel():
    global _CACHED
    if _CACHED is None:
        _CACHED = build_kernel()
    return _CACHED


def _prep_inputs(inputs, hidden, emb, Wx0, Wx, Wh, bh, Wout, bout):
    """Host-side sharding/layout prep. Returns list of 8 per-core input maps."""
    inputs = _np(inputs).astype(np.int32)     # [T, B]
    hidden = _np(hidden).astype(np.float32)   # [L, B, H]
    emb = np.ascontiguousarray(_np(emb), dtype=np.float32)
    Wx0 = _np(Wx0).astype(np.float32)         # [3, H, E]
    Wx = _np(Wx).astype(np.float32)           # [1, 3, H, H]
    Wh = _np(Wh).astype(np.float32)           # [L, 3, H, H]
    bh = _np(bh).astype(np.float32)           # [L, 3, H]
    Wout = _np(Wout).astype(np.float32)       # [V, H]
    bout = _np(bout).astype(np.float32)       # [V]

    # weight layouts (shared across cores)
    wx0T = np.concatenate([Wx0[g].T for g in range(3)], axis=1)       # [E, 3H]
    wx0T = np.concatenate([wx0T, bh[0].reshape(1, 3 * H)], axis=0)    # [E+1, 3H]
    wh0rzT = np.concatenate([Wh[0, 0].T, Wh[0, 1].T], axis=1)         # [H, 2H]
    wh0cT = np.ascontiguousarray(Wh[0, 2].T)
    wx1T = np.concatenate([Wx[0, g].T for g in range(3)], axis=1)     # [H, 3H]
    wx1T = np.concatenate([wx1T, bh[1].reshape(1, 3 * H)], axis=0)    # [H+1, 3H]
    wh1rzT = np.concatenate([Wh[1, 0].T, Wh[1, 1].T], axis=1)
    wh1cT = np.ascontiguousarray(Wh[1, 2].T)
    woutT = np.ascontiguousarray(Wout.T)                              # [H, V]

    in_maps = []
    for r in range(NC):
        bsl = slice(r * BL, (r + 1) * BL)
        # [T, BL] -> flat [TBL] (t-major), pad to GTILES*128, as [GTILES,128,1]
        idx = inputs[:, bsl].reshape(TBL)
        idxp = np.zeros(GTILES * 128, np.int32)
        idxp[:TBL] = idx
        in_maps.append({
            "idx_in": np.ascontiguousarray(idxp.reshape(GTILES, 128, 1)),
            "emb_in": emb,
            "wx0_in": np.ascontiguousarray(wx0T),
            "wh0rz_in": np.ascontiguousarray(wh0rzT),
            "wh0c_in": wh0cT,
            "wx1_in": np.ascontiguousarray(wx1T),
            "wh1rz_in": np.ascontiguousarray(wh1rzT),
            "wh1c_in": wh1cT,
            "wout_in": np.ascontiguousarray(woutT[:, r * VL:(r + 1) * VL]),
            "bout_in": np.ascontiguousarray(bout[r * VL:(r + 1) * VL].reshape(1, VL)),
            "hid_in": np.ascontiguousarray(hidden[:, bsl, :]),
        })
    return in_maps


def kernel(inputs, hidden, emb, Wx0, Wx, Wh, bh, Wout, bout, _trace=False):
    nc = _get_kernel()
    in_maps = _prep_inputs(inputs, hidden, emb, Wx0, Wx, Wh, bh, Wout, bout)
    res = run_bass_kernel_spmd(nc, in_maps, core_ids=list(range(NC)), trace=_trace)
    global LAST_EXEC_NS
    LAST_EXEC_NS = res.exec_time_ns

    logits = np.empty((T, B, V), np.float32)
    h_final = np.empty((L, B, H), np.float32)
    for r in range(NC):
        out = res.results[r]
        lg = out["logits_out"].reshape(NC, T, BL, VL)     # [rb, t, bl, vl]
        logits[:, :, r * VL:(r + 1) * VL] = \
            lg.transpose(1, 0, 2, 3).reshape(T, B, VL)
        h_final[:, r * BL:(r + 1) * BL, :] = out["hfin_out"]
    return logits, h_final


LAST_EXEC_NS = None
